# revision 34
# baseline (speedup 1.0000x reference)
"""Trainium2 Bass kernel for the SelfOrg spiking-network step.

Reference computation (per batch b, neuron n):
    z_out_new = BETA * z_out + z
    z_loo[b,j,n] = z_out_new[b, j + (j>=n)]            (leave-one-out gather)
    drive[b,n]  = sum_k x[b,k,n] * w[k,n]  (k < N_IN)
                + sum_j z_loo[b,j,n] * w[N_IN+j, n]
    v_new = ALPHA*v + drive - V_TH*z
    z_new = (v_new - V_TH > 0)

Strategy (v6 -- fp8e3 moving operand feeds the PE directly):
  * Neuron-sharded across 8 cores (64 neurons x 64 batches each). The
    PE accepts mixed-dtype matmuls (fp16 stationary w x fp8e3 moving
    x), so x bytes DMA'd from HBM feed the PE with ZERO dequant work.
  * 48 batches (8..31, 40..63) on the PE path: host encodes
    e3m4(x-0.5) (centering halves the quant step; the 0.5*sum_k w
    correction is folded into v host-side). e3m4 denormals verified
    exact on HW. 16 k-block matmuls per 8-batch group accumulate
    [64n x 512] into PSUM; 3 PSUM tiles hold 6 groups (2 per tile via
    tile_position halves). Diagonals are extracted per tile by one
    masked tensor_tensor multiply + one grouped tensor_reduce.
  * 16 batches (0..7, 32..39) on the DVE as u8: one stt per 2-batch
    pair computes (x*(1/255))*w with accum_out = the k-sum (~2.3us x
    8 pairs), balancing DVE (~23us) against PE (~22us).
  * Lateral term: zon = BETA*zo + z (DVE stt), 4 PE transposes, fp16
    cast on the otherwise-idle ACT engine, contracted with fp16 Wf as
    4 accumulating matmuls.
  * DMA: ~9.1MB/core on the two HWDGE rings. Tensors are merged
    (wk|wf, z|zo, v|zl, vn|zn, xg groups in one tensor) so each ring
    issues ~11 triggers (~0.6us each) in consumption-deadline order;
    x tiles are fully SBUF-resident so the stream never stalls.
"""

import numpy as np

# model hyperparameters (must match the reference)
N_IN = 2048
NN = 512
BATCH = 64
DT, TAU_M, TAU_X = 0.05, 10.0, 2.0
ALPHA = 1.0 - DT / TAU_M
BETA = 1.0 - DT / TAU_X
V_TH = 2.0

NCORES = 8
NLOC = NN // NCORES        # neurons per core (64)
NPAIR = 8                  # DVE batch pairs: pair c = batches (c, c+32)
NTILE = 3                  # PSUM tiles; tile i = groups (8+8i.., 40+8i..)
NGRP = 2 * NTILE           # 6 PE groups of 8 batches
NKB = N_IN // 128          # k-blocks (16)
GW = 8 * NLOC              # group moving width per k-block (512)
GB = NKB * GW              # bytes per group per partition row (8192)

# PE-path batches: xg column block s -> batches GBATCH[s]
PAIR_B = list(range(0, NPAIR)) + list(range(32, 32 + NPAIR))
GBATCH = []
for i in range(NTILE):
    GBATCH.append(list(range(8 + 8 * i, 16 + 8 * i)))      # tile i top
    GBATCH.append(list(range(40 + 8 * i, 48 + 8 * i)))     # tile i bottom
PE_B = [b for blk in GBATCH for b in blk]


def _build_nc():
    import concourse.mybir as mybir
    from concourse import bacc
    from concourse.masks import make_identity
    from concourse.tile import TileContext

    f32 = mybir.dt.float32
    f16 = mybir.dt.float16
    f8 = mybir.dt.float8e3
    u8 = mybir.dt.uint8
    AL = mybir.AluOpType
    nc = bacc.Bacc("TRN2", name="selforg_step")

    # pair path: xp[64h+n, (c, k)] = xq[c+32h, k, n0+n]  (8 pairs, u8)
    xp_h = nc.dram_tensor("xp", [128, NPAIR * N_IN], u8, kind="ExternalInput")
    # PE path, block s: xg[p, s*GB + (kb, j, n)] = e3m4(x-.5)[GBATCH[s][j], 128kb+p, n0+n]
    xg_h = nc.dram_tensor("xg", [128, NGRP * GB], f8, kind="ExternalInput")
    # wtkf = wt | wk | wf: wt[64h+n, k] = w[k, n0+n]; wk[p, (kb, m)] =
    # w[128kb+p, n0+m]; wf[p, (t, n)] = Wf[128t+p, n0+n]  (all fp16)
    wtkf_h = nc.dram_tensor(
        "wtkf", [128, N_IN + (NKB + 4) * NLOC], f16, kind="ExternalInput"
    )
    # zzovzl = z | zo (full neuron dim) | v | zl (local)
    zzovzl_h = nc.dram_tensor(
        "zzovzl", [BATCH, 2 * NN + 2 * NLOC], f32, kind="ExternalInput"
    )
    ovz_h = nc.dram_tensor("ovz", [BATCH, 2 * NLOC], f32, kind="ExternalOutput")
    ozon_h = nc.dram_tensor("ozon", [BATCH, NN], f32, kind="ExternalOutput")

    with TileContext(nc) as tc:
        with (
            tc.tile_pool(name="const", bufs=1) as cpool,
            tc.tile_pool(name="psg", bufs=1, space="PSUM") as ppoolg,
            tc.tile_pool(name="pslat", bufs=1, space="PSUM") as ppooll,
            tc.tile_pool(name="pstr", bufs=2, space="PSUM") as ppool2,
            tc.tile_pool(name="psT", bufs=1, space="PSUM") as ppoolT,
            tc.tile_pool(name="pswm", bufs=1, space="PSUM") as ppoolW,
        ):
            # ---- SBUF tiles ----
            zzovzl_sb = cpool.tile([BATCH, 2 * NN + 2 * NLOC], f32)
            wtkf_sb = cpool.tile([128, N_IN + (NKB + 4) * NLOC], f16)
            xp_sb = cpool.tile([128, NPAIR * N_IN], u8)
            xg_sb = cpool.tile([128, NGRP * GB], f8)
            zon_sb = cpool.tile([BATCH, NN], f32)
            zonT = cpool.tile([128, 4 * BATCH], f16)
            # acc_all[64h+n, c] = drive[c+32h, n]: cols 0..7 pairs, 8..31 PE
            acc_all = cpool.tile([128, 32], f32)
            scr = cpool.tile([128, N_IN], u8)      # stt junk product
            tmpx = cpool.tile([128, GW], f32)      # masked psg product
            identJ = cpool.tile([128, GW], f32)    # 8x tiled identity mask
            vz = cpool.tile([BATCH, 2 * NLOC], f32)  # [vn | zn]

            wt_sb = wtkf_sb[:, 0:N_IN]
            wk = wtkf_sb[:, N_IN : N_IN + NKB * NLOC]
            wf = wtkf_sb[:, N_IN + NKB * NLOC : N_IN + (NKB + 4) * NLOC]
            z_sb = zzovzl_sb[:, 0:NN]
            zo_sb = zzovzl_sb[:, NN : 2 * NN]
            v_sb = zzovzl_sb[:, 2 * NN : 2 * NN + NLOC]
            zl_sb = zzovzl_sb[:, 2 * NN + NLOC : 2 * NN + 2 * NLOC]

            # ---- DMA: trigger order = ring FIFO order ----
            def gdma(eng, s, frac=(0, 1), nfrac=1):
                a = s * GB + frac[0] * (GB // nfrac)
                b = s * GB + frac[1] * (GB // nfrac)
                eng.dma_start(xg_sb[:, a:b], xg_h[:, a:b])

            def pdma(eng, c0, c1):  # pairs [c0, c1)
                a, b = c0 * N_IN, c1 * N_IN
                eng.dma_start(xp_sb[:, a:b], xp_h[:, a:b])

            # Global consumption-deadline order, ~0.5MB chunks alternating
            # across the two HWDGE rings (each ring is FIFO; aggregate
            # fabric ~0.42 MB/us is the binding constraint).
            W = N_IN + (NKB + 4) * NLOC
            nc.scalar.dma_start(zzovzl_sb[:, :], zzovzl_h[:, :])        # SC1
            nc.sync.dma_start(wtkf_sb[:, N_IN:W], wtkf_h[:, N_IN:W])    # SY1 wk|wf
            nc.scalar.dma_start(                                        # SC2 wtB
                wtkf_sb[:, N_IN // 2 : N_IN], wtkf_h[:, N_IN // 2 : N_IN]
            )
            nc.sync.dma_start(                                          # SY2 wtA
                wtkf_sb[:, 0 : N_IN // 2], wtkf_h[:, 0 : N_IN // 2]
            )
            gdma(nc.scalar, 0, (0, 1), 2)                               # SC3 b0a
            pdma(nc.sync, 0, 2)                                         # SY3 xp01
            gdma(nc.scalar, 1, (0, 1), 2)                               # SC4 b1a
            gdma(nc.sync, 0, (1, 2), 2)                                 # SY4 b0b
            pdma(nc.scalar, 2, 4)                                       # SC5 xp23
            gdma(nc.sync, 1, (1, 2), 2)                                 # SY5 b1b
            gdma(nc.scalar, 2, (1, 2), 2)                               # SC6 b2b
            gdma(nc.sync, 2, (0, 1), 2)                                 # SY6 b2a
            gdma(nc.scalar, 3, (0, 1), 2)                               # SC7 b3a
            pdma(nc.sync, 4, 6)                                         # SY7 xp45
            gdma(nc.scalar, 4, (0, 1), 2)                               # SC8 b4a
            gdma(nc.sync, 3, (1, 2), 2)                                 # SY8 b3b
            gdma(nc.scalar, 4, (1, 2), 2)                               # SC9 b4b
            pdma(nc.sync, 6, 8)                                         # SY9 xp67
            gdma(nc.scalar, 5, (1, 2), 2)                               # SC10 b5b
            gdma(nc.sync, 5, (0, 1), 2)                                 # SY10 b5a

            # ---- identities / masks (gpsimd, off critical path) ----
            ident = cpool.tile([NLOC, NLOC], f32)
            make_identity(nc, ident[:, :])
            ident128 = cpool.tile([128, 128], f32)
            make_identity(nc, ident128[:, :])
            # identJ[64h+m, (j, n)] = 1 if m == n else 0
            nc.gpsimd.memset(identJ[:, :], 0.0)
            for hh in range(2):
                nc.gpsimd.affine_select(
                    out=identJ[64 * hh : 64 * hh + 64, :],
                    in_=identJ[64 * hh : 64 * hh + 64, :],
                    compare_op=mybir.AluOpType.not_equal,
                    fill=1.0,
                    base=0,
                    pattern=[[0, 8], [-1, NLOC]],
                    channel_multiplier=1,
                )

            # ---- PE path: 3 PSUM tiles x (top group, bottom group) ----
            psg = [
                ppoolg.tile([128, GW], f32, tag=f"g{i}", name=f"psg{i}")
                for i in range(NTILE)
            ]

            def do_group(i, half):
                s = 2 * i + half
                ps = psg[i]
                h0 = 64 * half
                for kb in range(NKB):
                    nc.tensor.matmul(
                        ps[h0 : h0 + 64, :],
                        wk[:, kb * NLOC : (kb + 1) * NLOC],
                        xg_sb[:, s * GB + kb * GW : s * GB + (kb + 1) * GW],
                        start=(kb == 0),
                        stop=(kb == NKB - 1),
                        tile_position=(0, h0),
                    )

            def pair_stt(c):
                nc.vector.scalar_tensor_tensor(
                    out=scr[:, :],
                    in0=xp_sb[:, c * N_IN : (c + 1) * N_IN],
                    scalar=1.0 / 255.0,
                    in1=wt_sb[:, :],
                    op0=AL.mult,
                    op1=AL.mult,
                    accum_out=acc_all[:, c : c + 1],
                )

            def extract_tile(i):
                # acc cols 8+8i..15+8i <- diagonals of psg[i] (both halves)
                nc.vector.tensor_tensor(
                    out=tmpx[:, :], in0=psg[i][:, :], in1=identJ[:, :],
                    op=AL.mult,
                )
                nc.vector.tensor_reduce(
                    out=acc_all[:, 8 + 8 * i : 16 + 8 * i],
                    in_=tmpx[:, :].rearrange("p (j n) -> p j n", j=8),
                    axis=mybir.AxisListType.X,
                    op=AL.add,
                )

            def do_zon_lat_pe():
                # 4 transposes of zon + 4 accumulating lat matmuls
                for t in range(4):
                    psum_t = ppool2.tile([128, BATCH], f32, tag="tr")
                    nc.tensor.transpose(
                        psum_t[:, :], zon_sb[:, t * 128 : (t + 1) * 128],
                        ident[:, :],
                    )
                    nc.scalar.activation(
                        out=zonT[:, t * BATCH : (t + 1) * BATCH],
                        in_=psum_t[:, :],
                        func=mybir.ActivationFunctionType.Copy,
                    )
                for t in range(4):
                    nc.tensor.matmul(
                        lat_tile[:, :],
                        zonT[:, t * BATCH : (t + 1) * BATCH],
                        wf[:, t * NLOC : (t + 1) * NLOC],
                        start=(t == 0),
                        stop=(t == 3),
                    )

            lat_tile = ppooll.tile([BATCH, NLOC], f32, tag="lat")

            # zon = BETA*zo + z: DVE opener (zzovzl is the first DMA chunk)
            nc.vector.scalar_tensor_tensor(
                out=zon_sb[:, :], in0=zo_sb[:, :], scalar=BETA, in1=z_sb[:, :],
                op0=AL.mult, op1=AL.add,
            )
            nc.scalar.dma_start(ozon_h[:, :], zon_sb[:, :])             # SC11

            # ---- main schedule (per-engine queues are in-order) ----
            # PE warmup: dep-free transposes flip the HAM clock gate to
            # 2.4GHz before the first real matmul arrives (~3us ramp)
            pswarm = ppoolW.tile([128, 128], f32, tag="warm")
            for _ in range(12):
                nc.tensor.transpose(pswarm[:, :], ident128[:, :], ident128[:, :])

            do_group(0, 0)
            do_zon_lat_pe()
            pair_stt(0)
            pair_stt(1)
            do_group(0, 1)
            pair_stt(2)
            pair_stt(3)
            do_group(1, 0)
            extract_tile(0)
            pair_stt(4)
            do_group(1, 1)
            pair_stt(5)
            do_group(2, 0)
            extract_tile(1)
            pair_stt(6)
            do_group(2, 1)
            pair_stt(7)

            # epilogue: pre = ALPHA*v + (lat - V_TH*zl)
            t2 = cpool.tile([BATCH, NLOC], f32)
            nc.vector.scalar_tensor_tensor(
                out=t2[:, :], in0=zl_sb[:, :], scalar=-V_TH, in1=lat_tile[:, :],
                op0=AL.mult, op1=AL.add,
            )
            pre = cpool.tile([BATCH, NLOC], f32)
            nc.vector.scalar_tensor_tensor(
                out=pre[:, :], in0=v_sb[:, :], scalar=ALPHA, in1=t2[:, :],
                op0=AL.mult, op1=AL.add,
            )

            extract_tile(2)

            # drive assembly fused with the final add: vn = psT + pre
            # psT[c, 64h+n] = drive[c+32h, n]
            psT = ppoolT.tile([32, 128], f32, tag="pT")
            nc.tensor.transpose(psT[:, :], acc_all[:, :], ident128[:, :])
            nc.vector.tensor_add(vz[0:32, 0:NLOC], psT[:, 0:NLOC], pre[0:32, :])
            nc.vector.tensor_add(vz[32:64, 0:NLOC], psT[:, NLOC:128], pre[32:64, :])
            nc.vector.tensor_scalar(
                out=vz[:, NLOC : 2 * NLOC], in0=vz[:, 0:NLOC],
                scalar1=V_TH, scalar2=None, op0=AL.is_gt,
            )
            nc.sync.dma_start(ovz_h[:, :], vz[:, :])                    # SY10

    return nc


def _make_wf(w: np.ndarray) -> np.ndarray:
    """Wf[m,n] = w[N_IN + m - (m>n), n] off-diagonal, 0 on the diagonal."""
    wl = w[N_IN:]
    m = np.arange(NN)[:, None]
    n = np.arange(NN)[None, :]
    idx = np.minimum(np.where(m > n, m - 1, m), NN - 2)
    return np.where(m == n, np.float32(0.0), wl[idx, n]).astype(np.float32)


def _make_in_maps(x, v, z, z_out, w):
    import ml_dtypes

    x = np.asarray(x, dtype=np.float32)
    v = np.ascontiguousarray(v, dtype=np.float32)
    z = np.ascontiguousarray(z, dtype=np.float32)
    z_out = np.ascontiguousarray(z_out, dtype=np.float32)
    w = np.asarray(w, dtype=np.float32)
    wf_full = _make_wf(w)
    w16 = w[:N_IN].astype(np.float16)

    # pair batches as u8; PE batches as e3m4(x - 0.5)
    xq = np.rint(x[PAIR_B] * 255.0).astype(np.uint8)          # (16, k, NN)
    xc8 = (x[PE_B] - 0.5).astype(ml_dtypes.float8_e3m4)       # (48, k, NN)

    # v correction for the centered PE batches: ALPHA*v' = ALPHA*v + .5*sum w
    wsum05 = 0.5 * w16.astype(np.float32).sum(axis=0)          # (NN,)

    in_maps = []
    for c in range(NCORES):
        sl = slice(c * NLOC, (c + 1) * NLOC)
        # pair path: pair c0 = batches (c0, c0+32), neurons on partitions
        xt = xq[:, :, sl].transpose(0, 2, 1)                   # (16, n, k)
        xp = np.zeros((128, NPAIR * N_IN), np.uint8)
        for c0 in range(NPAIR):
            xp[0:64, c0 * N_IN : (c0 + 1) * N_IN] = xt[c0]
            xp[64:128, c0 * N_IN : (c0 + 1) * N_IN] = xt[NPAIR + c0]
        # group path: block s at cols [s*GB, (s+1)*GB), layout (p, kb, j, n)
        xg = np.zeros((128, NGRP * GB), ml_dtypes.float8_e3m4)
        for s in range(NGRP):
            xs = xc8[8 * s : 8 * s + 8, :, sl]                 # (8, 2048, 64)
            xs = xs.reshape(8, NKB, 128, NLOC)                 # (j, kb, p, n)
            xg[:, s * GB : (s + 1) * GB] = np.ascontiguousarray(
                xs.transpose(2, 1, 0, 3)                       # (p, kb, j, n)
            ).reshape(128, GB)
        wsl = w16[:, sl]                                       # (k, n) fp16
        wt = np.tile(wsl.T, (2, 1))                            # (128, 2048)
        wk = np.ascontiguousarray(
            wsl.reshape(NKB, 128, NLOC).transpose(1, 0, 2)     # (p, kb, m)
        ).reshape(128, NKB * NLOC)
        wf16 = np.ascontiguousarray(
            wf_full[:, sl].astype(np.float16)
            .reshape(4, 128, NLOC).transpose(1, 0, 2)          # (p, t, n)
        ).reshape(128, 4 * NLOC)
        wtkf = np.concatenate([wt, wk, wf16], axis=1)
        vadj = v[:, sl].copy()
        vadj[PE_B] += wsum05[sl][None, :] / ALPHA
        zzovzl = np.concatenate([z, z_out, vadj, z[:, sl]], axis=1)
        in_maps.append(
            {
                "xp": np.ascontiguousarray(xp),
                "xg": np.ascontiguousarray(xg),
                "wtkf": np.ascontiguousarray(wtkf),
                "zzovzl": np.ascontiguousarray(zzovzl),
            }
        )
    return in_maps


def run(x, v, z, z_out, w, trace=False):
    """Build + run on the 8 NeuronCores; returns (output, BassKernelResults)."""
    from concourse.bass_utils import run_bass_kernel_spmd

    nc = _build_nc()
    if not nc.is_finalized():
        nc.finalize()
    in_maps = _make_in_maps(x, v, z, z_out, w)
    res = run_bass_kernel_spmd(nc, in_maps, core_ids=list(range(NCORES)), trace=trace)
    vn = np.concatenate([r["ovz"][:, 0:NLOC] for r in res.results], axis=1)
    zn = np.concatenate([r["ovz"][:, NLOC : 2 * NLOC] for r in res.results], axis=1)
    zon = res.results[0]["ozon"]
    full = np.stack([vn, zn, zon]).astype(np.float32)
    return np.ascontiguousarray(full), res


def kernel(x, v, z, z_out, w):
    out, _ = run(x, v, z, z_out, w)
    return out
